# revision 12
# baseline (speedup 1.0000x reference)
"""Trainium2 Bass kernel for GQA attention (QK-RMSNorm + RoPE + softmax + o_proj).

Collective-free sharding over 8 NeuronCores: core = (batch b in {0,1}, sequence
quarter sq in {0..3}).  Every core redundantly computes the FULL-batch K/V
(all 2048 positions, 4 kv heads) straight into SBUF — no AllGather, no DRAM
roundtrip, no inter-core barrier — then runs attention for its own 512 queries
and the o_proj, producing final output rows [512, 2048].

Per-core pipeline:
  1. K/V projection over the full batch, streamed per 512-position chunk:
     K^T accumulated in PSUM per kv head (RMSNorm+RoPE applied, bf16 into
     SBUF-resident KTs), V accumulated per 128-position tile (f32r into Vs).
  2. Q projection (16 heads) + RMSNorm + RoPE for the core's own 512 rows.
  3. scores^T tiles = K^T_tile.T @ Q^T, exp without max-subtraction (RMSNorm
     bounds |logit| <= sqrt(128) * max|w|^2), row-sums via DVE accumulation +
     ones-matmul, AV accumulation, 1/sum scaling.
  4. o_proj (contraction over all 16 heads) -> out rows [512, 2048].

Matmul operands are bf16 for x and all weights/activations except the
V / exp(scores) pair which stays float32r (full PE rate either way; bf16
halves SBUF footprint and HBM traffic).  Elementwise math stays float32.

kernel(**inputs) takes FULL unsharded inputs, returns the full output.
Host-side prep (weight transposes, bf16 casts, slicing) is numpy; device time
is the graded kernel.
"""
import contextlib

import numpy as np
import ml_dtypes

import concourse.bass as bass
import concourse.mybir as mybir
import concourse.tile as tile
from concourse import bacc
from concourse.bass_utils import run_bass_kernel_spmd

B, S, HID = 2, 2048, 2048
NH, NKV, D = 16, 4, 128
SC = 512           # per-core sequence chunk (queries)
KT_H = HID // 128  # 16 contraction tiles over hidden dim
NCH = S // SC      # 4 position chunks for full-batch K/V
EPS = 1e-6
INV_SQRT_D = 1.0 / float(np.sqrt(D))

F32R = mybir.dt.float32r
F32 = mybir.dt.float32
BF16 = mybir.dt.bfloat16
BF = ml_dtypes.bfloat16


def build_nc(mode="real", max_iters=64, probe=None):
    """mode: 'real' or 'timed' (body wrapped in a runtime-count For_i hardware
    loop; the body is collective-free so both modes emit identical work).
    probe: None | 'kts' | 'vs' | 'qt' — dump that intermediate to out."""
    nc = bacc.Bacc("TRN2", target_bir_lowering=False, debug=False, num_devices=8)

    d = {}
    d["xTc"] = nc.dram_tensor("xTc", [NCH * HID, SC], BF16, kind="ExternalInput")
    d["xqT"] = nc.dram_tensor("xqT", [HID, SC], BF16, kind="ExternalInput")
    d["wqT"] = nc.dram_tensor("wqT", [HID, NH * D], BF16, kind="ExternalInput")
    d["wkT"] = nc.dram_tensor("wkT", [HID, NKV * D], BF16, kind="ExternalInput")
    d["wvT"] = nc.dram_tensor("wvT", [HID, NKV * D], BF16, kind="ExternalInput")
    d["woT"] = nc.dram_tensor("woT", [NH * D, HID], BF16, kind="ExternalInput")
    d["cosf"] = nc.dram_tensor("cosf", [D, S], F32, kind="ExternalInput")
    d["sinf"] = nc.dram_tensor("sinf", [D, S], F32, kind="ExternalInput")
    d["cosq"] = nc.dram_tensor("cosq", [D, SC], F32, kind="ExternalInput")
    d["sinq"] = nc.dram_tensor("sinq", [D, SC], F32, kind="ExternalInput")
    d["qw"] = nc.dram_tensor("qw", [D, 1], F32, kind="ExternalInput")
    d["kw"] = nc.dram_tensor("kw", [D, 1], F32, kind="ExternalInput")
    d["rmat"] = nc.dram_tensor("rmat", [D, D], F32R, kind="ExternalInput")
    d["onesc"] = nc.dram_tensor("onesc", [128, 1], F32R, kind="ExternalInput")
    d["onesr"] = nc.dram_tensor("onesr", [1, 128], F32R, kind="ExternalInput")
    d["out"] = nc.dram_tensor("out", [SC, HID], F32, kind="ExternalOutput")
    if mode == "timed":
        d["nit"] = nc.dram_tensor("nit", [1, 128], mybir.dt.int32, kind="ExternalInput")

    with tile.TileContext(nc) as tc, \
         nc.allow_low_precision(reason="bf16/f32r matmuls within 2e-2 tolerance"):
        with contextlib.ExitStack() as ctx:
            cpool = ctx.enter_context(tc.tile_pool(name="consts", bufs=1))
            persist = ctx.enter_context(tc.tile_pool(name="persist", bufs=1))

            c = {}
            for nm, shape, dt_ in [("cosf", [D, S], F32), ("sinf", [D, S], F32),
                                   ("cosq", [D, SC], F32), ("sinq", [D, SC], F32),
                                   ("qw", [D, 1], F32), ("kw", [D, 1], F32),
                                   ("rmat", [D, D], F32R),
                                   ("onesc", [128, 1], F32R),
                                   ("onesr", [1, 128], F32R)]:
                c[nm] = cpool.tile(shape, dt_, name=f"c_{nm}")
                nc.sync.dma_start(out=c[nm][:], in_=d[nm][:])
            c["epsc"] = cpool.tile([128, 1], F32, name="c_epsc")
            nc.gpsimd.memset(c["epsc"][:], EPS)

            P = {
                "KTs": persist.tile([128, NKV * S], BF16, name="KTs"),
                "Vs": persist.tile([128, (S // 128) * 512], F32R, name="Vs"),
                "QT": persist.tile([128, NH * SC], BF16, name="QT"),
                "AVT": persist.tile([128, NH * SC], BF16, name="AVT"),
            }

            if mode == "timed":
                nit_sb = cpool.tile([1, 128], mybir.dt.int32)
                nc.sync.dma_start(out=nit_sb[:], in_=d["nit"][:])
                with tc.tile_critical():
                    regs = []
                    for e in mybir.ALL_ENGINES:
                        eng = nc.engines[e]
                        tmp = eng.alloc_register(f"nit_{e.name}")
                        eng.reg_load(tmp, nit_sb[0:1, 0:1])
                        regs.append(tmp)
                    n_val = nc.snap(bass.RegisterHandles(regs), donate=True,
                                    min_val=0, max_val=max_iters)
                loop_cm = tc.For_i(0, n_val, 1)
            else:
                loop_cm = contextlib.nullcontext()

            with loop_cm:
                _emit_body(nc, tc, d, c, P, probe=probe)
                if mode == "timed":
                    dummy = cpool.tile([1, 8], F32)
                    nc.gpsimd.memset(dummy[:], 0.0)

    nc.compile()
    return nc


def _emit_body(nc, tc, d, c, P, probe=None):
    KTs, Vs, QT, AVT = P["KTs"], P["Vs"], P["QT"], P["AVT"]

    def _dump(tile_ap):
        with tc.tile_pool(name="dump", bufs=1) as dp:
            for i in range(4):
                cp = dp.tile([128, HID], F32, tag="cp", bufs=2, name="cp")
                nc.vector.tensor_copy(cp[:], tile_ap[:, i * HID:(i + 1) * HID])
                nc.sync.dma_start(out=d["out"][i * 128:(i + 1) * 128, :],
                                  in_=cp[:])

    # ---------------- projections ----------------
    with contextlib.ExitStack() as ctx:
        w_pool = ctx.enter_context(tc.tile_pool(name="wkv", bufs=1))
        x_pool = ctx.enter_context(tc.tile_pool(name="xs", bufs=1))
        wsl_pool = ctx.enter_context(tc.tile_pool(name="wsl", bufs=1))
        rope_pool = ctx.enter_context(tc.tile_pool(name="rope", bufs=1))
        pp = ctx.enter_context(tc.tile_pool(name="pproj", bufs=1, space="PSUM"))

        def rope_block(psum_q, wcol, dst, dst_col, cosap, sinap):
            """RMSNorm(+w) and RoPE on psum_q [128 d, SC]; write to
            dst[:, dst_col:dst_col+SC] (dtype of dst)."""
            sq = rope_pool.tile([128, SC], F32R, tag="sq", bufs=2, name="sq")
            nc.scalar.square(sq[:], psum_q[:])
            psA = pp.tile([1, SC], F32, tag="pssum", bufs=1, name="psA")
            nc.tensor.matmul(psA[:], c["onesc"][:], sq[:], start=True, stop=True)
            rrow = rope_pool.tile([1, SC], F32, tag="rrow", bufs=1, name="rrow")
            nc.scalar.activation(rrow[:], psA[:], mybir.ActivationFunctionType.Sqrt,
                                 bias=c["epsc"][0:1, 0:1], scale=1.0 / D)
            rrec = rope_pool.tile([1, SC], F32R, tag="rrec", bufs=1, name="rrec")
            nc.vector.reciprocal(rrec[:], rrow[:])
            psR = pp.tile([128, SC], F32, tag="pr", bufs=1, name="psR")
            nc.tensor.matmul(psR[:], c["onesr"][:], rrec[:], start=True, stop=True)
            qw_t = rope_pool.tile([128, SC], F32R, tag="qwt", bufs=2, name="qwt")
            nc.vector.tensor_scalar_mul(qw_t[:], psum_q[:], wcol[:])
            prot = pp.tile([128, SC], F32, tag="prot", bufs=1, name="prot")
            nc.tensor.matmul(prot[:], c["rmat"][:], qw_t[:], start=True, stop=True)
            a_t = rope_pool.tile([128, SC], F32, tag="a", bufs=1, name="a_t")
            nc.vector.tensor_mul(a_t[:], qw_t[:], cosap)
            b_t = rope_pool.tile([128, SC], F32, tag="b", bufs=1, name="b_t")
            nc.vector.tensor_mul(b_t[:], prot[:], sinap)
            ab_t = rope_pool.tile([128, SC], F32, tag="ab", bufs=1, name="ab_t")
            nc.vector.tensor_add(ab_t[:], a_t[:], b_t[:])
            nc.vector.tensor_mul(dst[:, dst_col:dst_col + SC], ab_t[:], psR[:])

        # resident K/V weights (2 MB each, bf16), layout [128, kt*512 + col]
        wk_full = w_pool.tile([128, KT_H * NKV * D], BF16)
        nc.sync.dma_start(out=wk_full[:].rearrange("p (t c) -> p t c", t=KT_H),
                          in_=d["wkT"][:].rearrange("(t p) c -> p t c", p=128))
        wv_full = w_pool.tile([128, KT_H * NKV * D], BF16)
        nc.sync.dma_start(out=wv_full[:].rearrange("p (t c) -> p t c", t=KT_H),
                          in_=d["wvT"][:].rearrange("(t p) c -> p t c", p=128))

        if probe == "wkv0":
            with tc.tile_pool(name="dump", bufs=1) as dp:
                for i, (src, col) in enumerate([(wk_full, 0), (wk_full, HID),
                                                (wv_full, 0), (wv_full, HID)]):
                    cp = dp.tile([128, HID], F32, tag="cp", bufs=1, name="cp")
                    nc.vector.tensor_copy(cp[:], src[:, col:col + HID])
                    nc.sync.dma_start(out=d["out"][i * 128:(i + 1) * 128, :],
                                      in_=cp[:])
            return

        # full-batch K/V, one 512-position chunk at a time
        for ch in range(NCH):
            xc = x_pool.tile([128, KT_H * SC], BF16, tag="xc", bufs=2, name="xc")
            nc.sync.dma_start(
                out=xc[:].rearrange("p (t s) -> p t s", t=KT_H),
                in_=d["xTc"][ch * HID:(ch + 1) * HID, :]
                    .rearrange("(t p) s -> p t s", p=128))
            # K^T for 4 kv heads: psum [128 d, 512 pos], accumulate over kt
            for kvh in range(NKV):
                psk = pp.tile([128, SC], F32, tag="pq", bufs=4, name="psk")
                for kt in range(KT_H):
                    nc.tensor.matmul(
                        psk[:],
                        wk_full[:, kt * 512 + kvh * D: kt * 512 + (kvh + 1) * D],
                        xc[:, kt * SC:(kt + 1) * SC],
                        start=(kt == 0), stop=(kt == KT_H - 1))
                rope_block(psk, c["kw"], KTs, kvh * S + ch * SC,
                           c["cosf"][:, ch * SC:(ch + 1) * SC],
                           c["sinf"][:, ch * SC:(ch + 1) * SC])
            # V rows for 4 position tiles: psum [128 pos, 512 (kvh,dv)]
            for pt in range(4):
                psv = pp.tile([128, 512], F32, tag="pq", bufs=4, name="psv")
                for kt in range(KT_H):
                    nc.tensor.matmul(
                        psv[:],
                        xc[:, kt * SC + pt * 128: kt * SC + (pt + 1) * 128],
                        wv_full[:, kt * 512:(kt + 1) * 512],
                        start=(kt == 0), stop=(kt == KT_H - 1))
                nc.vector.tensor_copy(
                    Vs[:, (ch * 4 + pt) * 512:(ch * 4 + pt + 1) * 512], psv[:])

        # Q proj + norm/rope -> QT (heads in pairs; WqT streamed in slices)
        xq = x_pool.tile([128, KT_H * SC], BF16, tag="xc", bufs=2, name="xq")
        nc.sync.dma_start(
            out=xq[:].rearrange("p (t s) -> p t s", t=KT_H),
            in_=d["xqT"][:].rearrange("(t p) s -> p t s", p=128))
        for hp in range(NH // 2):
            wq_sl = wsl_pool.tile([128, KT_H * 2 * D], BF16, tag="wq", bufs=2,
                                  name="wq_sl")
            nc.sync.dma_start(
                out=wq_sl[:].rearrange("p (t c) -> p t c", t=KT_H),
                in_=d["wqT"][:, hp * 256:(hp + 1) * 256]
                    .rearrange("(t p) c -> p t c", p=128))
            for j in range(2):
                h = 2 * hp + j
                psq = pp.tile([128, SC], F32, tag="pq", bufs=4, name="psq")
                for kt in range(KT_H):
                    nc.tensor.matmul(
                        psq[:],
                        wq_sl[:, kt * 256 + j * D: kt * 256 + (j + 1) * D],
                        xq[:, kt * SC:(kt + 1) * SC],
                        start=(kt == 0), stop=(kt == KT_H - 1))
                rope_block(psq, c["qw"], QT, h * SC,
                           c["cosq"][:], c["sinq"][:])

        if probe == "wkv":
            with tc.tile_pool(name="dump", bufs=1) as dp:
                for i, (src, col) in enumerate([(wk_full, 0), (wk_full, HID),
                                                (wv_full, 0), (wv_full, HID)]):
                    cp = dp.tile([128, HID], F32, tag="cp", bufs=1, name="cp")
                    nc.vector.tensor_copy(cp[:], src[:, col:col + HID])
                    nc.sync.dma_start(out=d["out"][i * 128:(i + 1) * 128, :],
                                      in_=cp[:])
            return

    if probe == "kts":
        _dump(KTs)
        return
    if probe == "vs":
        _dump(Vs)
        return
    if probe == "qt":
        _dump(QT)
        return

    # ---------------- attention ----------------
    with contextlib.ExitStack() as ctx:
        pt_pool = ctx.enter_context(tc.tile_pool(name="pt", bufs=1))
        sm_pool = ctx.enter_context(tc.tile_pool(name="sm", bufs=1))
        pa = ctx.enter_context(tc.tile_pool(name="pattn", bufs=1, space="PSUM"))

        # process q-heads in pairs sharing the kv head: consecutive matmuls
        # share the stationary operand (K^T tile / V tile); exp row sums via
        # DVE-accumulated sum of the exp tiles + one ones-matmul per head.
        for grp in range(NH // 2):
            kvh = grp // 2
            h0 = 2 * grp
            pav = [pa.tile([128, SC], F32, tag=f"pavt{j}", bufs=1,
                           name=f"pav{j}") for j in range(2)]
            ptacc = sm_pool.tile([128, 2 * SC], F32, tag="ptacc", bufs=2,
                                 name="ptacc")
            for t in range(16):
                psc = pa.tile([128, 2 * SC], F32, tag="psc", bufs=2, name="psc")
                for j in range(2):
                    nc.tensor.matmul(
                        psc[:, j * SC:(j + 1) * SC],
                        KTs[:, kvh * S + t * 128: kvh * S + (t + 1) * 128],
                        QT[:, (h0 + j) * SC:(h0 + j + 1) * SC],
                        start=True, stop=True)
                pt_t = pt_pool.tile([128, 2 * SC], F32R, tag="pt", bufs=3,
                                    name="pt_t")
                nc.scalar.activation(pt_t[:], psc[:],
                                     mybir.ActivationFunctionType.Exp,
                                     bias=0.0, scale=INV_SQRT_D)
                for j in range(2):
                    nc.tensor.matmul(
                        pav[j][:],
                        Vs[:, t * 512 + kvh * D: t * 512 + (kvh + 1) * D],
                        pt_t[:, j * SC:(j + 1) * SC],
                        start=(t == 0), stop=(t == 15), skip_group_check=True)
                if t == 0:
                    nc.vector.tensor_copy(ptacc[:], pt_t[:])
                else:
                    nc.vector.tensor_add(ptacc[:], ptacc[:], pt_t[:])
            ptacc_r = sm_pool.tile([128, 2 * SC], F32R, tag="ptaccr", bufs=2,
                                   name="ptacc_r")
            nc.vector.tensor_copy(ptacc_r[:], ptacc[:])
            for j in range(2):
                h = h0 + j
                prow = pa.tile([1, SC], F32, tag="prow", bufs=1, name="prow")
                nc.tensor.matmul(prow[:], c["onesc"][:],
                                 ptacc_r[:, j * SC:(j + 1) * SC],
                                 start=True, stop=True)
                srec = sm_pool.tile([1, SC], F32R, tag="srec", bufs=2, name="srec")
                nc.vector.reciprocal(srec[:], prow[:])
                psR2 = pa.tile([128, SC], F32, tag="pr2", bufs=1, name="psR2")
                nc.tensor.matmul(psR2[:], c["onesr"][:], srec[:],
                                 start=True, stop=True)
                rb = sm_pool.tile([128, SC], F32, tag="rb", bufs=2, name="rb")
                nc.vector.tensor_copy(rb[:], psR2[:])
                nc.vector.tensor_mul(AVT[:, h * SC:(h + 1) * SC], pav[j][:], rb[:])

    # ---------------- o_proj ----------------
    with contextlib.ExitStack() as ctx:
        wo_pool = ctx.enter_context(tc.tile_pool(name="wo", bufs=1))
        oacc_pool = ctx.enter_context(tc.tile_pool(name="oacc", bufs=1))
        po_pool = ctx.enter_context(tc.tile_pool(name="po", bufs=1, space="PSUM"))

        out_acc = oacc_pool.tile([128, 4 * HID], F32)  # [s%128, st*HID + Hcol]
        for rnd in range(4):
            wo_ts = []
            for j in range(4):
                h = 4 * rnd + j
                wo_t = wo_pool.tile([128, HID], BF16, tag="wo", bufs=5,
                                    name=f"wo_t{h}")
                nc.sync.dma_start(out=wo_t[:], in_=d["woT"][h * 128:(h + 1) * 128, :])
                wo_ts.append(wo_t)
            for st in range(4):
                pos = [po_pool.tile([128, 512], F32, tag=f"po{hc}", bufs=2,
                                    name=f"po{hc}") for hc in range(4)]
                for j in range(4):
                    h = 4 * rnd + j
                    for hc in range(4):
                        nc.tensor.matmul(
                            pos[hc][:],
                            AVT[:, h * SC + st * 128: h * SC + (st + 1) * 128],
                            wo_ts[j][:, hc * 512:(hc + 1) * 512],
                            start=(j == 0), stop=(j == 3))
                for hc in range(4):
                    dst = out_acc[:, st * HID + hc * 512: st * HID + (hc + 1) * 512]
                    if rnd == 0:
                        nc.vector.tensor_copy(dst, pos[hc][:])
                    else:
                        nc.vector.tensor_add(dst, dst, pos[hc][:])
        for st in range(4):
            nc.sync.dma_start(out=d["out"][st * 128:(st + 1) * 128, :],
                              in_=out_acc[:, st * HID:(st + 1) * HID])


def host_prep(hidden_states, cos, sin, Wq, Wk, Wv, Wo, q_norm_w, k_norm_w):
    """Build the 8 per-core input maps (host-side layout prep)."""
    hs = np.asarray(hidden_states, dtype=np.float32)
    cos = np.asarray(cos, dtype=np.float32)
    sin = np.asarray(sin, dtype=np.float32)
    sinp = np.concatenate([-sin[..., :64], sin[..., 64:]], axis=-1)
    wqT = np.ascontiguousarray(np.asarray(Wq, np.float32).T).astype(BF)
    wkT = np.ascontiguousarray(np.asarray(Wk, np.float32).T).astype(BF)
    wvT = np.ascontiguousarray(np.asarray(Wv, np.float32).T).astype(BF)
    woT = np.ascontiguousarray(np.asarray(Wo, np.float32).T).astype(BF)
    rmat = np.zeros((D, D), np.float32)
    rmat[(np.arange(D) + 64) % D, np.arange(D)] = 1.0
    onesc = np.ones((128, 1), np.float32)
    onesr = np.ones((1, 128), np.float32)
    qwc = np.asarray(q_norm_w, np.float32).reshape(D, 1)
    kwc = np.asarray(k_norm_w, np.float32).reshape(D, 1)

    # per batch: x^T [hid, S] in chunk-major rows [ch*HID + h, pos], bf16
    xTc_b, cosf_b, sinf_b = [], [], []
    for b in range(B):
        xT = hs[b].T  # [HID, S]
        xTc = np.ascontiguousarray(
            xT.reshape(HID, NCH, SC).transpose(1, 0, 2).reshape(NCH * HID, SC)
        ).astype(BF)
        xTc_b.append(xTc)
        cosf_b.append(np.ascontiguousarray(cos[b].T))
        sinf_b.append(np.ascontiguousarray(sinp[b].T))

    in_maps = []
    for core in range(8):
        b, sq = divmod(core, 4)
        in_maps.append({
            "xTc": xTc_b[b],
            "xqT": np.ascontiguousarray(xTc_b[b][sq * HID:(sq + 1) * HID, :]),
            "wqT": wqT, "wkT": wkT, "wvT": wvT, "woT": woT,
            "cosf": cosf_b[b], "sinf": sinf_b[b],
            "cosq": np.ascontiguousarray(cosf_b[b][:, sq * SC:(sq + 1) * SC]),
            "sinq": np.ascontiguousarray(sinf_b[b][:, sq * SC:(sq + 1) * SC]),
            "qw": qwc, "kw": kwc,
            "rmat": rmat, "onesc": onesc, "onesr": onesr,
        })
    return in_maps


_nc_cache = {}


def get_nc(mode="real"):
    if mode not in _nc_cache:
        _nc_cache[mode] = build_nc(mode)
    return _nc_cache[mode]


def kernel(**inputs) -> np.ndarray:
    nc = get_nc("real")
    in_maps = host_prep(**inputs)
    res = run_bass_kernel_spmd(nc, in_maps, list(range(8)))
    out = np.empty((B, S, HID), np.float32)
    for core in range(8):
        b, sq = divmod(core, 4)
        out[b, sq * SC:(sq + 1) * SC, :] = res.results[core]["out"]
    return out


if __name__ == "__main__":
    import reference
    inputs = {k: np.asarray(v) for k, v in reference.setup_inputs().items()}
    expected = np.asarray(reference.reference(**inputs))
    actual = kernel(**inputs)
    err = np.abs(actual - expected)
    rel = err.max() / np.abs(expected).max()
    print(f"max abs err {err.max():.3e}  rel (vs absmax) {rel:.3e}")
